# revision 38
# baseline (speedup 1.0000x reference)
"""Transformer-XL CompressiveLayer on 8 TRN2 NeuronCores.

Sharding: core c = (batch b = c//2) x (head-half hh = c%2).
Each core handles one batch's full 896 tokens with 8 of 16 heads and
2048 of 4096 FFN inner channels.  One pairwise exchange (bf16) joins the
attention output halves before the post-LN; the FFN partial outputs are
summed on the host (plus ff2 bias).

LN1's affine is folded on the host: gamma scales the rows of Wq/Wk/Wv,
beta folds into the q biases (rwb/rrb), beta@Wk cancels in softmax (a
per-row constant scales num and den equally), and beta + beta@Wv@Wo is
added as a per-core constant during the o-projection copy.  On device
LN1 is just (x-mu)*rstd; the residual term xhat*gamma is produced
in-place during P1 slack.

The attention front-end runs in fp8e4m3 (weights x8 host-side, xhat and
pos quantized on device/host), which numerically costs ~1e-2 relative
error (vs the 2e-2 budget) and enables DoubleRow matmuls:
  - q/k/v/r projections and the o-projection contract two 128-deep
    k-tiles per DoubleRow instruction,
  - the AC/BD score matmuls use a stride-0 "both slots read the same
    data" DoubleRow (result doubled; folded into the exp scale).
All score-side tensors carry a 128x scale (8x q * 8x k * 2x DoubleRow)
that the exp scale (s/128) and the 8.0 ones-column of the PV stationary
matrix absorb exactly.  The FFN and PV stay bf16 (fp8 there fails the
error budget).

Attention scores: P = exp(s*(AC + BD_shifted)).  BD is written per head
as two strided DMAs into a padded DRAM buffer (row stride K+1, col 0
preset) and read back as ONE DMA-transpose, which yields the
rel-shifted term in [j, i] layout.  Heads run in one of two modes:
exp-product (exp writes, P = expAC*expBD via a DVE multiply) or
identity-add (raw BD written, accumulated into the AC PSUM via an
identity matmul, single exp) - identity-add moves elementwise work off
the saturated Act engine.  The PV stationary matrix carries an appended
8.0-column making row 64 of the PV output the softmax denominator; a
1-partition reciprocal + broadcast matmul normalizes after PV.
"""

import numpy as np
import ml_dtypes
from contextlib import ExitStack

import concourse.bass as bass
import concourse.tile as tile
from concourse.tile import add_dep_helper
from concourse import mybir, bacc
from concourse.bass_utils import run_bass_kernel_spmd

F32 = mybir.dt.float32
BF16 = mybir.dt.bfloat16
E4 = mybir.dt.float8e4
BF = ml_dtypes.bfloat16
E4NP = ml_dtypes.float8_e4m3
DR = mybir.MatmulPerfMode.DoubleRow

D, H, DH, FF = 1024, 16, 64, 4096
S, B, M, CM = 512, 4, 256, 128
K = S + M + CM          # 896 tokens
NT = K // 128           # 7 token tiles
ND = D // 128           # 8 D chunks
HC = 8                  # heads per core
FC = HC * DH            # 512 features per core
NFC = FC // 128         # 4 feature chunks per core
FFC = FF // 2           # 2048 ffn channels per core
NFF = FFC // 128        # 16 ffn chunks per core
SCALE = 1.0 / np.sqrt(DH)
WS = 8.0                # host-side fp8 weight scale
EXPS = float(SCALE / (2 * WS * WS))   # exp scale absorbing 8x*8x*2(DR)
EPS = 1e-5
PVLAG = 2               # heads of pipeline depth between S^T and PV
ID_HEADS = frozenset((4, 5, 6, 7))  # tail heads identity-add

_CACHED = {}


def _dr0(a, n=2):
    """Insert a stride-0 double-read dim after the partition dim."""
    return bass.AP(tensor=a.tensor, offset=a.offset,
                   ap=[list(a.ap[0]), [0, n]] + [list(x) for x in a.ap[1:]])


def build(debug=False, sim_mode=False):
    nc = bacc.Bacc(None)

    xb = nc.declare_dram_parameter("xb", [128, NT, D], BF16, isOutput=False)
    posT = nc.declare_dram_parameter("posT", [128, NT, ND, 128], E4,
                                     isOutput=False)
    qw = nc.declare_dram_parameter("qw", [128, NFC, ND, 128], E4,
                                   isOutput=False)
    kw = nc.declare_dram_parameter("kw", [128, NFC, ND, 128], E4,
                                   isOutput=False)
    vw = nc.declare_dram_parameter("vw", [128, ND, FC], E4, isOutput=False)
    rw = nc.declare_dram_parameter("rw", [128, ND, FC], E4, isOutput=False)
    ow = nc.declare_dram_parameter("ow", [128, NFC, D], E4, isOutput=False)
    ff1w = nc.declare_dram_parameter("ff1w", [128, ND, FFC], BF16,
                                     isOutput=False)
    ff2w = nc.declare_dram_parameter("ff2w", [128, NFF, D], BF16,
                                     isOutput=False)
    rwb = nc.declare_dram_parameter("rwb", [128, NFC], F32, isOutput=False)
    rrb = nc.declare_dram_parameter("rrb", [128, NFC], F32, isOutput=False)
    ff1b = nc.declare_dram_parameter("ff1b", [128, NFF], F32, isOutput=False)
    gam = nc.declare_dram_parameter("gam", [D], BF16, isOutput=False)
    cc = nc.declare_dram_parameter("cc", [D], BF16, isOutput=False)
    ident = nc.declare_dram_parameter("ident", [128, 128], BF16,
                                      isOutput=False)

    out = nc.declare_dram_parameter("out", [128, NT, D], BF16, isOutput=True)

    pads = [nc.dram_tensor(f"pad{h}", [K * (K + 1)], BF16) for h in range(HC)]
    CH = [(0, 1), (1, 1), (2, 3), (5, 2)]   # exchange/FFN1 chunking
    ao_bufs = [(nc.dram_tensor(f"ao_in{i}", [128, n, D], BF16),
                nc.dram_tensor(f"ao_out{i}", [128, n, D], BF16))
               for i, (t0, n) in enumerate(CH)]

    with tile.TileContext(nc) as tc, ExitStack() as ctx:
        consts = ctx.enter_context(tc.tile_pool(name="consts", bufs=1))
        stat_pool = ctx.enter_context(tc.tile_pool(name="stats", bufs=8))
        # PSUM (8 banks): psA 2x2 (exp-paced bd/st stream + FFN1),
        # psB 1x2 (projections, bc, FFN2-even), psC 1x2 (PV, FFN2-odd)
        psA = ctx.enter_context(tc.tile_pool(name="psA", bufs=2, space="PSUM"))
        psB = ctx.enter_context(tc.tile_pool(name="psB", bufs=1, space="PSUM"))
        psC = ctx.enter_context(tc.tile_pool(name="psC", bufs=1, space="PSUM"))
        rrctr = [0]

        def ps_rr(pools=(psA, psA, psB)):
            rrctr[0] += 1
            ps = pools[rrctr[0] % len(pools)].tile([128, 1024], F32,
                                                   tag="s", name="ps")
            return ps

        eps_sb = consts.tile([128, 1], F32)
        nc.vector.memset(eps_sb, EPS)
        ones_row = consts.tile([128, NT], BF16)
        nc.vector.memset(ones_row, 1.0)
        zero_row = consts.tile([128, NT], BF16)
        nc.vector.memset(zero_row, 0.0)
        ones64 = consts.tile([1, 64], BF16)
        nc.vector.memset(ones64, 1.0)
        rwb_sb = consts.tile([128, NFC], F32)
        rrb_sb = consts.tile([128, NFC], F32)
        ff1b_sb = consts.tile([128, NFF], F32)
        ident_sb = consts.tile([128, 128], BF16)

        def bcast(name, param):
            t = consts.tile([128, D], BF16, tag=name)
            nc.gpsimd.dma_start(out=t, in_=bass.AP(tensor=param, offset=0,
                                                   ap=[[0, 128], [1, D]]))
            return t

        # persists through P2/P3
        persist = ctx.enter_context(tc.tile_pool(name="persist", bufs=1))
        ff1p = ctx.enter_context(tc.tile_pool(name="ff1p", bufs=1))
        ff1w_sb = ff1p.tile([128, ND, FFC], BF16)
        w_sb = persist.tile([128, NT, D], BF16)      # xhat, later xhat*gamma
        avT = persist.tile([128, NFC, K], E4)
        ow_sb = persist.tile([128, NFC, D], E4)
        ao_sb = persist.tile([128, NT, D], BF16)
        # wTf8 + per-fc q/k weight slices: live P0..end of P1
        p0_stack = ExitStack()
        p0b = p0_stack.enter_context(tc.tile_pool(name="p0b", bufs=1))
        wTf8 = p0b.tile([128, NT, ND, 128], E4)
        mv_all = p0b.tile([128, NT, 2], F32)
        rstds = p0b.tile([128, NT], F32)
        qkp = p0_stack.enter_context(tc.tile_pool(name="qkp", bufs=2))
        wtp = p0_stack.enter_context(tc.tile_pool(name="wtp", bufs=2))
        # attention tensors: live P0..end of P1 only
        attn_stack = ExitStack()
        attn = attn_stack.enter_context(tc.tile_pool(name="attn", bufs=1))
        q1T = attn.tile([128, NFC, K], E4)           # 8*(q + r_w_bias)^T
        q2T = attn.tile([128, NFC, K], E4)           # 8*(q + r_r_bias)^T
        kT = attn.tile([128, NFC, K], E4)            # 8*k^T
        rT = attn.tile([128, NFC, K], E4)            # 8*r^T
        v1 = attn.tile([128, NT, HC, DH + 1], BF16)  # 8*v + 8.0 column
        nc.vector.memset(v1[:, :, :, DH:DH + 1], WS)

        # ---------------- P0: loads, LN1, transpose, projections ---------
        # early buffers: dead once rT/vproj are emitted
        p0a_stack = ExitStack()
        p0w = p0a_stack.enter_context(tc.tile_pool(name="p0w", bufs=1))
        xb_sb = p0w.tile([128, NT, D], BF16)
        vw_sb = p0w.tile([128, ND, FC], E4)
        rw_sb = p0w.tile([128, ND, FC], E4)
        posT_sb = p0w.tile([128, NT, ND, 128], E4)

        # DMA issue order sets DMA_ENGINES service order (roughly):
        # xb[0:4] -> rw -> posT[0:4] -> vw -> posT[4:7] -> xb[4:7];
        # smalls/bcasts on the Pool queue; qw/kw stream per-fc later
        nc.gpsimd.dma_start(out=rwb_sb, in_=rwb[:])
        nc.gpsimd.dma_start(out=rrb_sb, in_=rrb[:])
        nc.gpsimd.dma_start(out=ff1b_sb, in_=ff1b[:])
        nc.gpsimd.dma_start(out=ident_sb, in_=ident[:])
        gam_bc = bcast("gam_bc", gam)
        cc_bc = bcast("cc_bc", cc)
        nc.sync.dma_start(out=xb_sb[:, 0:2, :], in_=xb[:, 0:2, :])
        nc.sync.dma_start(out=xb_sb[:, 2:4, :], in_=xb[:, 2:4, :])
        nc.scalar.dma_start(out=rw_sb, in_=rw[:])
        nc.scalar.dma_start(out=posT_sb[:, 0:4, :, :], in_=posT[:, 0:4, :, :])
        nc.scalar.dma_start(out=vw_sb, in_=vw[:])
        nc.scalar.dma_start(out=posT_sb[:, 4:NT, :, :],
                            in_=posT[:, 4:NT, :, :])
        nc.sync.dma_start(out=xb_sb[:, 4:NT, :], in_=xb[:, 4:NT, :])
        # rel-shift pad column: exp(0)=1 for exp-product heads, 0 for
        # identity-add heads (additive identity)
        for h, pad in enumerate(pads):
            nc.gpsimd.dma_start(
                out=bass.AP(tensor=pad, offset=0,
                            ap=[[K + 1, 128], [(K + 1) * 128, NT]]),
                in_=zero_row if h in ID_HEADS else ones_row)

        def ln1_stats(t):
            stats = stat_pool.tile([128, 2, 6], F32, tag="stats")
            for g in range(2):
                nc.vector.bn_stats(out=stats[:, g, :],
                                   in_=xb_sb[:, t, g * 512:(g + 1) * 512])
            nc.vector.bn_aggr(out=mv_all[:, t, :], in_=stats)

        def ln1_sqrt(t0, t1):
            nc.scalar.activation(
                out=rstds[:, t0:t1], in_=mv_all[:, t0:t1, 1],
                func=mybir.ActivationFunctionType.Sqrt,
                bias=eps_sb, scale=1.0)
            nc.vector.reciprocal(out=rstds[:, t0:t1], in_=rstds[:, t0:t1])

        def ln1_norm(t):
            # xhat only; gamma/beta folded into the projection weights
            nc.vector.tensor_scalar(out=w_sb[:, t, :], in0=xb_sb[:, t, :],
                                    scalar1=mv_all[:, t, 0:1],
                                    scalar2=rstds[:, t:t + 1],
                                    op0=mybir.AluOpType.subtract,
                                    op1=mybir.AluOpType.mult)
            wt_t = wtp.tile([128, ND, 128], BF16, tag="wt")
            nc.sync.dma_start_transpose(wt_t, w_sb[:, t, :])
            nc.scalar.copy(out=wTf8[:, t, :, :], in_=wt_t)

        def vproj(t):
            pv = ps_rr((psA, psA, psB, psC))
            for dp in range(ND // 2):
                nc.tensor.matmul(
                    pv[:, 0:512], wTf8[:, t, 2 * dp:2 * dp + 2, :],
                    vw_sb[:, 2 * dp:2 * dp + 2, :],
                    start=(dp == 0), stop=(dp == ND // 2 - 1), perf_mode=DR)
            nc.vector.tensor_copy(
                out=v1[:, t, :, 0:DH],
                in_=pv[:, 0:512].rearrange("p (h d) -> p h d", h=HC))

        def projT_fc(w_fc, dst, fc, bias1=None, dst2=None, bias1b=None,
                     rhs_sb=None, pool=None, grp=None, act_copy=False):
            # dst[., fc, i] = (W[:, fc-block].T @ w^T)[feat, tok], DoubleRow
            # over dc pairs.  grp: None = both column groups, "a" = tokens
            # 0:512 (tiles 0-3), "b" = tokens 512:896 (tiles 4-6)
            ps = ps_rr(pool) if isinstance(pool, tuple) else \
                (pool or psB).tile([128, 1024], F32, tag="s")
            full = w_fc.shape[-1] == FC
            for dp in range(ND // 2):
                st = dict(start=(dp == 0), stop=(dp == ND // 2 - 1))
                lh = (w_fc[:, 2 * dp:2 * dp + 2, fc * 128:(fc + 1) * 128]
                      if full else w_fc[:, 2 * dp:2 * dp + 2, :])
                r = rhs_sb if rhs_sb is not None else wTf8
                if grp != "b":
                    nc.tensor.matmul(
                        ps[:, 0:512],
                        lh, r[:, 0:4, 2 * dp:2 * dp + 2, :]
                        .rearrange("p t c f -> p c t f"),
                        perf_mode=DR, **st)
                if grp != "a":
                    nc.tensor.matmul(
                        ps[:, 512:896],
                        lh, r[:, 4:7, 2 * dp:2 * dp + 2, :]
                        .rearrange("p t c f -> p c t f"),
                        perf_mode=DR, **st)
            c0, c1 = (0, K) if grp is None else \
                ((0, 512) if grp == "a" else (512, K))
            if bias1 is not None:
                nc.vector.tensor_scalar_add(
                    out=dst[:, fc, c0:c1], in0=ps[:, c0:c1],
                    scalar1=bias1[:, fc:fc + 1])
                nc.vector.tensor_scalar_add(
                    out=dst2[:, fc, c0:c1], in0=ps[:, c0:c1],
                    scalar1=bias1b[:, fc:fc + 1])
            elif act_copy:
                nc.scalar.copy(out=dst[:, fc, c0:c1], in_=ps[:, c0:c1])
            else:
                nc.vector.tensor_copy(out=dst[:, fc, c0:c1],
                                      in_=ps[:, c0:c1])

        # LN1 pipeline: tiles 0-3 first, then rT group a fills the PE
        # while xb[4:7]/LN-b complete
        ln1_stats(0)
        ln1_stats(1)
        ln1_sqrt(0, 2)
        ln1_norm(0)
        ln1_norm(1)
        ln1_stats(2)
        ln1_stats(3)
        ln1_sqrt(2, 4)
        ln1_norm(2)
        ln1_norm(3)
        for fc in range(NFC):
            projT_fc(rw_sb, rT, fc, rhs_sb=posT_sb,
                     pool=(psA, psA, psB, psC), grp="a",
                     act_copy=(fc % 2 == 0))
        for t in range(4):
            vproj(t)
        for t in range(4, NT):
            ln1_stats(t)
        ln1_sqrt(4, NT)
        for t in range(4, NT):
            ln1_norm(t)
        for fc in range(NFC):
            projT_fc(rw_sb, rT, fc, rhs_sb=posT_sb,
                     pool=(psA, psA, psB, psC), grp="b",
                     act_copy=(fc % 2 == 0))
        for t in range(4, NT):
            vproj(t)
        p0a_stack.close()

        # ---------------- P1: attention ----------------
        bd_pool = attn_stack.enter_context(tc.tile_pool(name="bdp", bufs=2))
        sbdT_pool = attn_stack.enter_context(
            tc.tile_pool(name="sbdp", bufs=2))
        pT_pool = attn_stack.enter_context(tc.tile_pool(name="ptp", bufs=2))
        eac_pool = attn_stack.enter_context(tc.tile_pool(name="eacp", bufs=2))
        rden_pool = attn_stack.enter_context(tc.tile_pool(name="rdnp", bufs=2))

        def bd_phase(h):
            fc, hp = h // 2, (h % 2) * DH
            id_mode = h in ID_HEADS
            bd_all = bd_pool.tile([128, NT, K], BF16, tag="bd")
            for it in range(NT):
                ps = psA.tile([128, 1024], F32, tag="s")
                lh = _dr0(q2T[hp:hp + DH, fc, it * 128:(it + 1) * 128])
                nc.tensor.matmul(ps[:, 0:512],
                                 lh, _dr0(rT[hp:hp + DH, fc, 0:512]),
                                 start=True, stop=True, perf_mode=DR)
                nc.tensor.matmul(ps[:, 512:896],
                                 lh, _dr0(rT[hp:hp + DH, fc, 512:896]),
                                 start=True, stop=True, perf_mode=DR)
                if id_mode:
                    # raw 128x-scaled BD; copy alternates DVE/Act
                    if it % 2 == 0:
                        nc.vector.tensor_copy(out=bd_all[:, it, :],
                                              in_=ps[:, 0:K])
                    else:
                        nc.scalar.copy(out=bd_all[:, it, :], in_=ps[:, 0:K])
                else:
                    nc.scalar.activation(out=bd_all[:, it, :], in_=ps[:, 0:K],
                                         func=mybir.ActivationFunctionType.Exp,
                                         scale=EXPS)
            # pads split 5+2 tiles so the first sbdT transpose half
            # (reads rows <= 640) can start before the tail tiles land
            nc.gpsimd.dma_start(
                out=bass.AP(tensor=pads[h], offset=1,
                            ap=[[K + 1, 128], [(K + 1) * 128, 5], [1, K]]),
                in_=bd_all[:, 0:5, :])
            return nc.gpsimd.dma_start(
                out=bass.AP(tensor=pads[h], offset=(K + 1) * 128 * 5 + 1,
                            ap=[[K + 1, 128], [(K + 1) * 128, 2], [1, K]]),
                in_=bd_all[:, 5:NT, :])

        def st_phase(h):
            # sbdT[j, jt, i] = shifted BD term [i, jt*128+j] via DMA
            # transpose of the padded buffer read at flat offset K + K*i + j.
            fc, hp = h // 2, (h % 2) * DH
            id_mode = h in ID_HEADS
            sbdT = sbdT_pool.tile([128, NT, K], BF16, tag="sbdT")
            tp = nc.sync.dma_start_transpose(
                sbdT, bass.AP(tensor=pads[h], offset=K,
                              ap=[[K, K], [1, K]]))
            pT_all = pT_pool.tile([128, NT, K], BF16, tag="pT")
            for jt in range(NT):
                ps = psA.tile([128, 1024], F32, tag="s")
                lh = _dr0(kT[hp:hp + DH, fc, jt * 128:(jt + 1) * 128])
                if id_mode:
                    nc.tensor.matmul(ps[:, 0:512], lh,
                                     _dr0(q1T[hp:hp + DH, fc, 0:512]),
                                     start=True, stop=False, perf_mode=DR)
                    nc.tensor.matmul(ps[:, 512:896], lh,
                                     _dr0(q1T[hp:hp + DH, fc, 512:896]),
                                     start=True, stop=False, perf_mode=DR)
                    nc.tensor.matmul(ps[:, 0:512], ident_sb,
                                     sbdT[:, jt, 0:512],
                                     start=False, stop=True)
                    nc.tensor.matmul(ps[:, 512:896], ident_sb,
                                     sbdT[:, jt, 512:896],
                                     start=False, stop=True)
                else:
                    nc.tensor.matmul(ps[:, 0:512], lh,
                                     _dr0(q1T[hp:hp + DH, fc, 0:512]),
                                     start=True, stop=True, perf_mode=DR)
                    nc.tensor.matmul(ps[:, 512:896], lh,
                                     _dr0(q1T[hp:hp + DH, fc, 512:896]),
                                     start=True, stop=True, perf_mode=DR)
                nc.scalar.activation(out=pT_all[:, jt, :], in_=ps[:, 0:K],
                                     func=mybir.ActivationFunctionType.Exp,
                                     scale=EXPS)
            if not id_mode:
                for jt in range(NT):
                    nc.vector.tensor_mul(out=pT_all[:, jt, :],
                                         in0=pT_all[:, jt, :],
                                         in1=sbdT[:, jt, :])
            return pT_all, tp

        def pv_phase(h, pT_all):
            fc, hp = h // 2, (h % 2) * DH
            pvx = psC.tile([128, 1024], F32, tag="s")
            for jt in range(NT):
                st = dict(start=(jt == 0), stop=(jt == NT - 1))
                lh = v1[:, jt, h, :]
                nc.tensor.matmul(pvx[0:DH + 1, 0:512], lh,
                                 pT_all[:, jt, 0:512], **st)
                nc.tensor.matmul(pvx[0:DH + 1, 512:896], lh,
                                 pT_all[:, jt, 512:896], **st)
            rden = rden_pool.tile([1, K], BF16, tag="rden")
            with nc.allow_low_precision(reason="softmax denom recip in bf16"):
                nc.vector.reciprocal(out=rden, in_=pvx[DH:DH + 1, 0:K])
            bc = psB.tile([128, 1024], F32, tag="s")
            nc.tensor.matmul(bc[0:DH, 0:512], ones64, rden[:, 0:512],
                             start=True, stop=True)
            nc.tensor.matmul(bc[0:DH, 512:896], ones64, rden[:, 512:896],
                             start=True, stop=True)
            bc_sb = eac_pool.tile([DH, K], BF16, tag="bc_sb")
            nc.vector.tensor_copy(out=bc_sb, in_=bc[0:DH, 0:K])
            nc.vector.tensor_mul(out=avT[hp:hp + DH, fc, :],
                                 in0=pvx[0:DH, 0:K], in1=bc_sb)

        def poa_phase():
            # o_proj partial (fc 0,1 = heads 0..3) + per-core constant
            # (beta + beta@Wv@Wo fold); fills the P1 tail.  ow is x8 in
            # fp8, folded back by the 1/8 in the copy.
            for it in range(NT):
                po = (psB if it % 2 else psC).tile([128, 1024], F32, tag="s")
                l = avT[:, 0:2, it * 128:(it + 1) * 128]
                nc.tensor.matmul(po[:, 0:512], l, ow_sb[:, 0:2, 0:512],
                                 start=True, stop=True, perf_mode=DR)
                nc.tensor.matmul(po[:, 512:1024], l, ow_sb[:, 0:2, 512:1024],
                                 start=True, stop=True, perf_mode=DR)
                nc.vector.scalar_tensor_tensor(
                    out=ao_sb[:, it, :], in0=po, scalar=1.0 / WS,
                    in1=cc_bc, op0=mybir.AluOpType.mult,
                    op1=mybir.AluOpType.add)

        # heads interleave with the q/k projection chunks they depend on;
        # FFN weights stream on the SP queue behind each head's sbdT
        pending = {}
        ffq = ([("ow", c) for c in range(2)]
               + [("ff1", dc) for dc in range(ND)])

        def stream_ffw(n, anchor):
            for _ in range(n):
                if not ffq:
                    return
                kind, c = ffq.pop(0)
                if kind == "ow":
                    d = nc.sync.dma_start(out=ow_sb[:, 2 * c:2 * c + 2, :],
                                          in_=ow[:, 2 * c:2 * c + 2, :])
                else:
                    d = nc.sync.dma_start(out=ff1w_sb[:, c, :],
                                          in_=ff1w[:, c, :])
                if anchor is not None:
                    add_dep_helper(d.ins, anchor.ins, sync=True,
                                   reason="stream ff weights behind P1")

        def load_qk(fc):
            qw_fc = qkp.tile([128, ND, 128], E4, tag="qw")
            nc.gpsimd.dma_start(out=qw_fc, in_=qw[:, fc, :, :])
            kw_fc = qkp.tile([128, ND, 128], E4, tag="kw")
            nc.gpsimd.dma_start(out=kw_fc, in_=kw[:, fc, :, :])
            return qw_fc, kw_fc

        qk_bufs = [load_qk(0), load_qk(1)]
        for fc in range(NFC):
            qw_fc, kw_fc = qk_bufs[fc % 2]
            pp = psB
            projT_fc(qw_fc, q1T, fc, bias1=rwb_sb, dst2=q2T, bias1b=rrb_sb,
                     pool=pp)
            projT_fc(kw_fc, kT, fc, pool=pp, act_copy=True)
            if fc + 2 < NFC:
                qk_bufs[fc % 2] = load_qk(fc + 2)
            h0, h1 = 2 * fc, 2 * fc + 1
            bd_phase(h0)
            if h0 >= PVLAG:
                pv_phase(h0 - PVLAG, pending.pop(h0 - PVLAG)[0])
            bd_phase(h1)
            if h0 < NT:
                nc.vector.tensor_mul(out=w_sb[:, h0, :],
                                     in0=w_sb[:, h0, :], in1=gam_bc)
            pending[h0] = st_phase(h0)
            stream_ffw(2 if h0 < 4 else 1, pending[h0][1])
            if h1 >= PVLAG:
                pv_phase(h1 - PVLAG, pending.pop(h1 - PVLAG)[0])
            if h1 < NT:
                nc.vector.tensor_mul(out=w_sb[:, h1, :],
                                     in0=w_sb[:, h1, :], in1=gam_bc)
            pending[h1] = st_phase(h1)
            stream_ffw(2 if h1 < 4 else 1, pending[h1][1])
        stream_ffw(len(ffq), None)
        for h in range(HC - PVLAG, HC):
            pv_phase(h, pending.pop(h)[0])
        poa_phase()

        attn_stack.close()
        p0_stack.close()

        # ---------------- P2+P3: o_proj 2nd half, exchange, LN2, FFN ----
        p23w = ctx.enter_context(tc.tile_pool(name="p23w", bufs=1))
        ff2w_sb = p23w.tile([128, NFF, D], BF16)
        hT = p23w.tile([128, NFF, K], BF16)
        aor_sb = p23w.tile([128, NT, D], BF16)
        ars_all = aor_sb  # LN2 output overwrites the exchanged ao in place
        arT = p23w.tile([128, NT, ND, 128], BF16)

        with tc.tile_pool(name="p2", bufs=2) as p2:
            for ci, (t0, nt_c) in enumerate(CH):
                ain, aout = ao_bufs[ci]
                if ci == 1:
                    # ff2w streams early on the SP queue (behind chunk 0's
                    # exchange DMAs, ahead of the later chunks')
                    for g in range(4):
                        nc.sync.dma_start(
                            out=ff2w_sb[:, 4 * g:4 * g + 4, :],
                            in_=ff2w[:, 4 * g:4 * g + 4, :])
                for it in range(t0, t0 + nt_c):
                    po = (psB if it % 2 else psC).tile([128, 1024], F32,
                                                       tag="s")
                    l = avT[:, 2:4, it * 128:(it + 1) * 128]
                    nc.tensor.matmul(po[:, 0:512], l, ow_sb[:, 2:4, 0:512],
                                     start=True, stop=True, perf_mode=DR)
                    nc.tensor.matmul(po[:, 512:1024], l,
                                     ow_sb[:, 2:4, 512:1024],
                                     start=True, stop=True, perf_mode=DR)
                    nc.vector.scalar_tensor_tensor(
                        out=ao_sb[:, it, :], in0=po, scalar=1.0 / WS,
                        in1=ao_sb[:, it, :], op0=mybir.AluOpType.mult,
                        op1=mybir.AluOpType.add)
                nc.scalar.dma_start(out=ain[:], in_=ao_sb[:, t0:t0 + nt_c, :])
                if sim_mode:
                    nc.gpsimd.dma_start(out=aout[:], in_=ain[:])
                else:
                    nc.gpsimd.collective_compute(
                        "AllReduce", mybir.AluOpType.add,
                        replica_groups=[[0, 1], [2, 3], [4, 5], [6, 7]],
                        ins=[ain[:]], outs=[aout[:]])
                nc.sync.dma_start(out=aor_sb[:, t0:t0 + nt_c, :], in_=aout[:])
                for it in range(t0, t0 + nt_c):
                    x2 = p2.tile([128, D], BF16, tag="x2")
                    nc.vector.tensor_add(out=x2, in0=w_sb[:, it, :],
                                         in1=aor_sb[:, it, :])
                    stats = stat_pool.tile([128, 2, 6], F32, tag="stats")
                    for g in range(2):
                        nc.vector.bn_stats(out=stats[:, g, :],
                                           in_=x2[:, g * 512:(g + 1) * 512])
                    mv = stat_pool.tile([128, 2], F32, tag="mv")
                    nc.vector.bn_aggr(out=mv, in_=stats)
                    rstd = stat_pool.tile([128, 1], F32, tag="rstd")
                    nc.scalar.activation(
                        out=rstd, in_=mv[:, 1:2],
                        func=mybir.ActivationFunctionType.Sqrt,
                        bias=eps_sb, scale=1.0)
                    nc.vector.reciprocal(out=rstd, in_=rstd)
                    # LN2 affine is folded into ff1w/ff1b on the host
                    nc.vector.tensor_scalar(out=ars_all[:, it, :], in0=x2,
                                            scalar1=mv[:, 0:1], scalar2=rstd,
                                            op0=mybir.AluOpType.subtract,
                                            op1=mybir.AluOpType.mult)
                nc.sync.dma_start_transpose(arT[:, t0:t0 + nt_c, :, :],
                                            ars_all[:, t0:t0 + nt_c, :])
                # FFN1 for this chunk
                n = nt_c * 128
                for ffc in range(NFF):
                    ph = psA.tile([128, 1024], F32, tag="s")
                    for dc in range(ND):
                        st = dict(start=(dc == 0), stop=(dc == ND - 1))
                        lh = ff1w_sb[:, dc, ffc * 128:(ffc + 1) * 128]
                        nc.tensor.matmul(ph[:, 0:n], lh,
                                         arT[:, t0:t0 + nt_c, dc, :], **st)
                    nc.scalar.activation(
                        out=hT[:, ffc, t0 * 128:t0 * 128 + n], in_=ph[:, 0:n],
                        func=mybir.ActivationFunctionType.Relu,
                        bias=ff1b_sb[:, ffc:ffc + 1], scale=1.0)

        # ---------------- FFN2 ----------------
        with tc.tile_pool(name="p3", bufs=3) as p3:
            for it in range(NT):
                po = (psB if it % 2 else psC).tile([128, 1024], F32, tag="s")
                if it < NT - 1:
                    for ffc in range(NFF):
                        st = dict(start=(ffc == 0), stop=(ffc == NFF - 1))
                        l = hT[:, ffc, it * 128:(it + 1) * 128]
                        nc.tensor.matmul(po[:, 0:512], l,
                                         ff2w_sb[:, ffc, 0:512], **st)
                        nc.tensor.matmul(po[:, 512:1024], l,
                                         ff2w_sb[:, ffc, 512:1024], **st)
                    ot = p3.tile([128, D], BF16, tag="ot")
                    nc.vector.tensor_copy(out=ot, in_=po)
                    nc.sync.dma_start(out=out[:, it, :], in_=ot)
                else:
                    # last tile: accumulate+drain per quarter so early
                    # quarters' copy+DMA overlap later quarters' matmuls
                    for qi, c0 in enumerate((0, 256, 512, 768)):
                        poq = psA.tile([128, 1024], F32, tag="s")
                        for ffc in range(NFF):
                            st = dict(start=(ffc == 0), stop=(ffc == NFF - 1))
                            l = hT[:, ffc, it * 128:(it + 1) * 128]
                            nc.tensor.matmul(poq[:, 0:256], l,
                                             ff2w_sb[:, ffc, c0:c0 + 256],
                                             **st)
                        oth = p3.tile([128, 256], BF16, tag="oth")
                        nc.vector.tensor_copy(out=oth, in_=poq[:, 0:256])
                        nc.sync.dma_start(out=out[:, it, c0:c0 + 256],
                                          in_=oth)

    nc.finalize()
    return nc


def prep_inputs(inputs):
    """Full inputs -> list of 8 per-core input maps."""
    x_nat = np.concatenate([inputs["input_ids"], inputs["mem"],
                            inputs["c_mem"]], axis=0)  # [K,B,D] f32

    def to8(x, scale=1.0):
        return np.clip(np.asarray(x, np.float32) * scale,
                       -240, 240).astype(E4NP)

    # posT[p, t, dc, l] = pos[t*128 + l, dc*128 + p], fp8
    posn = np.asarray(inputs["positional_embedding"], np.float32)
    posT_t = np.ascontiguousarray(
        to8(posn).reshape(NT, 128, ND, 128).transpose(3, 0, 2, 1))

    qkv = np.asarray(inputs["qkv_w"], np.float32)
    gamma = np.asarray(inputs["ln1_w"], np.float32)
    beta = np.asarray(inputs["ln1_b"], np.float32)
    # fold LN1 affine: w = xhat*gamma + beta
    Wq = qkv[:, 0 * H * DH:1 * H * DH]
    Wk = qkv[:, 1 * H * DH:2 * H * DH]
    Wv = qkv[:, 2 * H * DH:3 * H * DH]
    Wq_g, Wk_g, Wv_g = (gamma[:, None] * Wq, gamma[:, None] * Wk,
                        gamma[:, None] * Wv)
    cq = beta @ Wq          # [H*DH] folds into both q biases
    cv = beta @ Wv          # [H*DH] folds into the o-proj constant
    # beta @ Wk adds a per-row constant to the logits -> cancels in softmax
    rwb_eff = np.asarray(inputs["r_w_bias"], np.float32).reshape(-1) + cq
    rrb_eff = np.asarray(inputs["r_r_bias"], np.float32).reshape(-1) + cq

    # fold LN2's affine into the FFN first layer
    ln2w_f = np.asarray(inputs["ln2_w"], np.float32)
    ln2b_f = np.asarray(inputs["ln2_b"], np.float32)
    ff1w_eff = np.asarray(inputs["ff1_w"], np.float32) * ln2w_f[:, None]
    ff1b_eff = (np.asarray(inputs["ff1_b"], np.float32)
                + ln2b_f @ np.asarray(inputs["ff1_w"], np.float32))
    o_w = np.asarray(inputs["o_w"], np.float32)
    ident_np = np.eye(128, dtype=np.float32).astype(BF)
    maps = []
    for c in range(8):
        b, hh = c // 2, c % 2
        Fs = slice(hh * FC, (hh + 1) * FC)
        FFs = slice(hh * FFC, (hh + 1) * FFC)
        # per-core o-proj constant: cv@Wo for this half + beta/2 (the
        # pairwise allreduce sums both halves -> cv@Wo full + beta)
        cc_core = cv[Fs] @ o_w[Fs, :] + 0.5 * beta

        def wchunk8(wmat):  # [D, FC] -> [128, ND, FC] fp8 x8
            return np.ascontiguousarray(
                to8(wmat, WS).reshape(ND, 128, -1).transpose(1, 0, 2))

        def wchunk(wmat):   # [D, F] -> [128, ND, F] bf16
            return np.ascontiguousarray(
                wmat.astype(BF).reshape(ND, 128, -1).transpose(1, 0, 2))

        m = {
            "xb": np.ascontiguousarray(
                x_nat[:, b, :].astype(BF).reshape(NT, 128, D)
                .transpose(1, 0, 2)),
            "posT": posT_t,
            # fc-major fp8: [128, NFC, ND, 128]
            "qw": np.ascontiguousarray(
                wchunk8(Wq_g[:, Fs]).reshape(128, ND, NFC, 128)
                .transpose(0, 2, 1, 3)),
            "kw": np.ascontiguousarray(
                wchunk8(Wk_g[:, Fs]).reshape(128, ND, NFC, 128)
                .transpose(0, 2, 1, 3)),
            "vw": wchunk8(Wv_g[:, Fs]),
            "rw": wchunk8(np.asarray(inputs["r_w"], np.float32)[:, Fs]),
            "ow": np.ascontiguousarray(
                to8(o_w[Fs, :], WS)
                .reshape(NFC, 128, D).transpose(1, 0, 2)),
            "ff1w": wchunk(ff1w_eff[:, FFs]),
            "ff2w": np.ascontiguousarray(
                np.asarray(inputs["ff2_w"], np.float32)[FFs, :].astype(BF)
                .reshape(NFF, 128, D).transpose(1, 0, 2)),
            # q biases are added to the 8x-scaled q: upload x8
            "rwb": np.ascontiguousarray(
                (WS * rwb_eff[hh * FC:(hh + 1) * FC])
                .reshape(NFC, 128).T.astype(np.float32)),
            "rrb": np.ascontiguousarray(
                (WS * rrb_eff[hh * FC:(hh + 1) * FC])
                .reshape(NFC, 128).T.astype(np.float32)),
            "ff1b": np.ascontiguousarray(
                ff1b_eff[FFs].reshape(NFF, 128).T.astype(np.float32)),
            "gam": gamma.astype(BF),
            "cc": cc_core.astype(BF),
            "ident": ident_np,
        }
        maps.append(m)
    return maps


class PjrtRunner:
    """Persistent jitted SPMD executor for a prebuilt Bass module."""

    def __init__(self, nc, n_cores=8):
        import jax
        from jax.sharding import Mesh, PartitionSpec
        from jax.experimental.shard_map import shard_map
        from concourse import mybir as _mybir
        from concourse.bass2jax import (_bass_exec_p, install_neuronx_cc_hook,
                                        partition_id_tensor)
        install_neuronx_cc_hook()
        self.jax = jax
        self.n_cores = n_cores
        in_names, out_names, out_avals = [], [], []
        partition_name = (nc.partition_id_tensor.name
                          if nc.partition_id_tensor else None)
        for alloc in nc.m.functions[0].allocations:
            if not isinstance(alloc, _mybir.MemoryLocationSet):
                continue
            name = alloc.memorylocations[0].name
            if alloc.kind == "ExternalInput":
                if name != partition_name:
                    in_names.append(name)
            elif alloc.kind == "ExternalOutput":
                out_names.append(name)
                out_avals.append(jax.core.ShapedArray(
                    tuple(alloc.tensor_shape), _mybir.dt.np(alloc.dtype)))
        self.in_names, self.out_names, self.out_avals = \
            in_names, out_names, out_avals

        def _body(*args):
            operands = list(args)
            if partition_name is not None:
                operands.append(partition_id_tensor())
            all_in = in_names + out_names
            if partition_name is not None:
                all_in = all_in + [partition_name]
            return tuple(_bass_exec_p.bind(
                *operands,
                out_avals=tuple(out_avals),
                in_names=tuple(all_in),
                out_names=tuple(out_names),
                lowering_input_output_aliases=(),
                sim_require_finite=True,
                sim_require_nnan=True,
                nc=nc,
            ))

        devices = jax.devices()[:n_cores]
        self.mesh = Mesh(np.asarray(devices), ("core",))
        nin = len(in_names) + len(out_names)
        self.fn = jax.jit(shard_map(
            _body, mesh=self.mesh,
            in_specs=(PartitionSpec("core"),) * nin,
            out_specs=(PartitionSpec("core"),) * len(out_names),
            check_rep=False))

    def pack(self, maps):
        arrs = [self.jax.device_put(
                    np.concatenate([np.asarray(maps[c][n])
                                    for c in range(self.n_cores)], axis=0))
                for n in self.in_names]
        arrs += [self.jax.device_put(
                    np.zeros((self.n_cores * a.shape[0], *a.shape[1:]),
                             a.dtype))
                 for a in self.out_avals]
        return arrs

    def __call__(self, packed):
        return self.fn(*packed)

    def unpack(self, outs):
        res = []
        for c in range(self.n_cores):
            res.append({
                n: np.asarray(outs[i]).reshape(
                    self.n_cores, *self.out_avals[i].shape)[c]
                for i, n in enumerate(self.out_names)})
        return res


def get_runner(debug=False, sim_mode=False):
    key = (bool(debug), bool(sim_mode))
    if key not in _CACHED:
        nc = build(debug=debug, sim_mode=sim_mode)
        _CACHED[key] = PjrtRunner(nc, 8)
    return _CACHED[key]


def _unpack_out(arr):
    # [128, NT, D] -> [K, D]
    return np.asarray(arr, np.float32).transpose(1, 0, 2).reshape(K, D)


def _assemble(inputs, results):
    ff2b = np.asarray(inputs["ff2_b"], np.float32)
    out = np.zeros((K, B, D), np.float32)
    for b in range(B):
        out[:, b, :] = (_unpack_out(results[2 * b]["out"])
                        + _unpack_out(results[2 * b + 1]["out"])
                        + ff2b[None, :])
    return out


def run(inputs, trace=False, debug=False):
    runner = get_runner(debug=False)
    maps = prep_inputs(inputs)
    packed = runner.pack(maps)
    outs = runner(packed)
    results = runner.unpack(outs)

    class R:
        pass
    res = R()
    res.results = results
    res.exec_time_ns = None
    return _assemble(inputs, results), res


def kernel(**inputs):
    inputs = {k: np.asarray(v) for k, v in inputs.items()}
    out, _ = run(inputs, trace=False, debug=False)
    return out


# revision 39
# speedup vs baseline: 1.0034x; 1.0034x over previous
"""Transformer-XL CompressiveLayer on 8 TRN2 NeuronCores.

Sharding: core c = (batch b = c//2) x (head-half hh = c%2).
Each core handles one batch's full 896 tokens with 8 of 16 heads and
2048 of 4096 FFN inner channels.  One pairwise exchange (bf16) joins the
attention output halves before the post-LN; the FFN partial outputs are
summed on the host (plus ff2 bias).

LN1's affine is folded on the host: gamma scales the rows of Wq/Wk/Wv,
beta folds into the q biases (rwb/rrb), beta@Wk cancels in softmax (a
per-row constant scales num and den equally), and beta + beta@Wv@Wo is
added as a per-core constant during the o-projection copy.  On device
LN1 is just (x-mu)*rstd; the residual term xhat*gamma is produced
in-place during P1 slack.

The attention front-end runs in fp8e4m3 (weights x8 host-side, xhat and
pos quantized on device/host), which numerically costs ~1e-2 relative
error (vs the 2e-2 budget) and enables DoubleRow matmuls:
  - q/k/v/r projections and the o-projection contract two 128-deep
    k-tiles per DoubleRow instruction,
  - the AC/BD score matmuls use a stride-0 "both slots read the same
    data" DoubleRow (result doubled; folded into the exp scale).
All score-side tensors carry a 128x scale (8x q * 8x k * 2x DoubleRow)
that the exp scale (s/128) and the 8.0 ones-column of the PV stationary
matrix absorb exactly.  The FFN and PV stay bf16 (fp8 there fails the
error budget).

Attention scores: P = exp(s*(AC + BD_shifted)).  BD is written per head
as two strided DMAs into a padded DRAM buffer (row stride K+1, col 0
preset) and read back as ONE DMA-transpose, which yields the
rel-shifted term in [j, i] layout.  Heads run in one of two modes:
exp-product (exp writes, P = expAC*expBD via a DVE multiply) or
identity-add (raw BD written, accumulated into the AC PSUM via an
identity matmul, single exp) - identity-add moves elementwise work off
the saturated Act engine.  The PV stationary matrix carries an appended
8.0-column making row 64 of the PV output the softmax denominator; a
1-partition reciprocal + broadcast matmul normalizes after PV.
"""

import numpy as np
import ml_dtypes
from contextlib import ExitStack

import concourse.bass as bass
import concourse.tile as tile
from concourse.tile import add_dep_helper
from concourse import mybir, bacc
from concourse.bass_utils import run_bass_kernel_spmd

F32 = mybir.dt.float32
BF16 = mybir.dt.bfloat16
E4 = mybir.dt.float8e4
BF = ml_dtypes.bfloat16
E4NP = ml_dtypes.float8_e4m3
DR = mybir.MatmulPerfMode.DoubleRow

D, H, DH, FF = 1024, 16, 64, 4096
S, B, M, CM = 512, 4, 256, 128
K = S + M + CM          # 896 tokens
NT = K // 128           # 7 token tiles
ND = D // 128           # 8 D chunks
HC = 8                  # heads per core
FC = HC * DH            # 512 features per core
NFC = FC // 128         # 4 feature chunks per core
FFC = FF // 2           # 2048 ffn channels per core
NFF = FFC // 128        # 16 ffn chunks per core
SCALE = 1.0 / np.sqrt(DH)
WS = 8.0                # host-side fp8 weight scale
EXPS = float(SCALE / (2 * WS * WS))   # exp scale absorbing 8x*8x*2(DR)
EPS = 1e-5
PVLAG = 2               # heads of pipeline depth between S^T and PV
ID_HEADS = frozenset()  # exp-product everywhere

_CACHED = {}


def _dr0(a, n=2):
    """Insert a stride-0 double-read dim after the partition dim."""
    return bass.AP(tensor=a.tensor, offset=a.offset,
                   ap=[list(a.ap[0]), [0, n]] + [list(x) for x in a.ap[1:]])


def build(debug=False, sim_mode=False):
    nc = bacc.Bacc(None)

    xb = nc.declare_dram_parameter("xb", [128, NT, D], BF16, isOutput=False)
    posT = nc.declare_dram_parameter("posT", [128, NT, ND, 128], E4,
                                     isOutput=False)
    qw = nc.declare_dram_parameter("qw", [128, NFC, ND, 128], E4,
                                   isOutput=False)
    kw = nc.declare_dram_parameter("kw", [128, NFC, ND, 128], E4,
                                   isOutput=False)
    vw = nc.declare_dram_parameter("vw", [128, ND, FC], E4, isOutput=False)
    rw = nc.declare_dram_parameter("rw", [128, ND, FC], E4, isOutput=False)
    ow = nc.declare_dram_parameter("ow", [128, NFC, D], E4, isOutput=False)
    ff1w = nc.declare_dram_parameter("ff1w", [128, ND, FFC], BF16,
                                     isOutput=False)
    ff2w = nc.declare_dram_parameter("ff2w", [128, NFF, D], BF16,
                                     isOutput=False)
    rwb = nc.declare_dram_parameter("rwb", [128, NFC], F32, isOutput=False)
    rrb = nc.declare_dram_parameter("rrb", [128, NFC], F32, isOutput=False)
    ff1b = nc.declare_dram_parameter("ff1b", [128, NFF], F32, isOutput=False)
    gam = nc.declare_dram_parameter("gam", [D], BF16, isOutput=False)
    cc = nc.declare_dram_parameter("cc", [D], BF16, isOutput=False)
    ident = nc.declare_dram_parameter("ident", [128, 128], BF16,
                                      isOutput=False)

    out = nc.declare_dram_parameter("out", [128, NT, D], BF16, isOutput=True)

    pads = [nc.dram_tensor(f"pad{h}", [K * (K + 1)], BF16) for h in range(HC)]
    CH = [(0, 1), (1, 1), (2, 3), (5, 2)]   # exchange/FFN1 chunking
    ao_bufs = [(nc.dram_tensor(f"ao_in{i}", [128, n, D], BF16),
                nc.dram_tensor(f"ao_out{i}", [128, n, D], BF16))
               for i, (t0, n) in enumerate(CH)]

    with tile.TileContext(nc) as tc, ExitStack() as ctx:
        consts = ctx.enter_context(tc.tile_pool(name="consts", bufs=1))
        stat_pool = ctx.enter_context(tc.tile_pool(name="stats", bufs=8))
        # PSUM (8 banks): psA 2x2 (exp-paced bd/st stream + FFN1),
        # psB 1x2 (projections, bc, FFN2-even), psC 1x2 (PV, FFN2-odd)
        psA = ctx.enter_context(tc.tile_pool(name="psA", bufs=2, space="PSUM"))
        psB = ctx.enter_context(tc.tile_pool(name="psB", bufs=1, space="PSUM"))
        psC = ctx.enter_context(tc.tile_pool(name="psC", bufs=1, space="PSUM"))
        rrctr = [0]

        def ps_rr(pools=(psA, psA, psB)):
            rrctr[0] += 1
            ps = pools[rrctr[0] % len(pools)].tile([128, 1024], F32,
                                                   tag="s", name="ps")
            return ps

        eps_sb = consts.tile([128, 1], F32)
        nc.vector.memset(eps_sb, EPS)
        ones_row = consts.tile([128, NT], BF16)
        nc.vector.memset(ones_row, 1.0)
        zero_row = consts.tile([128, NT], BF16)
        nc.vector.memset(zero_row, 0.0)
        ones64 = consts.tile([1, 64], BF16)
        nc.vector.memset(ones64, 1.0)
        rwb_sb = consts.tile([128, NFC], F32)
        rrb_sb = consts.tile([128, NFC], F32)
        ff1b_sb = consts.tile([128, NFF], F32)
        ident_sb = consts.tile([128, 128], BF16)

        def bcast(name, param):
            t = consts.tile([128, D], BF16, tag=name)
            nc.gpsimd.dma_start(out=t, in_=bass.AP(tensor=param, offset=0,
                                                   ap=[[0, 128], [1, D]]))
            return t

        # persists through P2/P3
        persist = ctx.enter_context(tc.tile_pool(name="persist", bufs=1))
        ff1p = ctx.enter_context(tc.tile_pool(name="ff1p", bufs=1))
        ff1w_sb = ff1p.tile([128, ND, FFC], BF16)
        w_sb = persist.tile([128, NT, D], BF16)      # xhat, later xhat*gamma
        avT = persist.tile([128, NFC, K], E4)
        ow_sb = persist.tile([128, NFC, D], E4)
        ao_sb = persist.tile([128, NT, D], BF16)
        # wTf8 + per-fc q/k weight slices: live P0..end of P1
        p0_stack = ExitStack()
        p0b = p0_stack.enter_context(tc.tile_pool(name="p0b", bufs=1))
        wTf8 = p0b.tile([128, NT, ND, 128], E4)
        mv_all = p0b.tile([128, NT, 2], F32)
        rstds = p0b.tile([128, NT], F32)
        qkp = p0_stack.enter_context(tc.tile_pool(name="qkp", bufs=2))
        wtp = p0_stack.enter_context(tc.tile_pool(name="wtp", bufs=2))
        # attention tensors: live P0..end of P1 only
        attn_stack = ExitStack()
        attn = attn_stack.enter_context(tc.tile_pool(name="attn", bufs=1))
        q1T = attn.tile([128, NFC, K], E4)           # 8*(q + r_w_bias)^T
        q2T = attn.tile([128, NFC, K], E4)           # 8*(q + r_r_bias)^T
        kT = attn.tile([128, NFC, K], E4)            # 8*k^T
        rT = attn.tile([128, NFC, K], E4)            # 8*r^T
        v1 = attn.tile([128, NT, HC, DH + 1], BF16)  # 8*v + 8.0 column
        nc.vector.memset(v1[:, :, :, DH:DH + 1], WS)

        # ---------------- P0: loads, LN1, transpose, projections ---------
        # early buffers: dead once rT/vproj are emitted
        p0a_stack = ExitStack()
        p0w = p0a_stack.enter_context(tc.tile_pool(name="p0w", bufs=1))
        xb_sb = p0w.tile([128, NT, D], BF16)
        vw_sb = p0w.tile([128, ND, FC], E4)
        rw_sb = p0w.tile([128, ND, FC], E4)
        posT_sb = p0w.tile([128, NT, ND, 128], E4)

        # DMA issue order sets DMA_ENGINES service order (roughly):
        # xb[0:4] -> rw -> posT[0:4] -> vw -> posT[4:7] -> xb[4:7];
        # smalls/bcasts on the Pool queue; qw/kw stream per-fc later
        nc.gpsimd.dma_start(out=rwb_sb, in_=rwb[:])
        nc.gpsimd.dma_start(out=rrb_sb, in_=rrb[:])
        nc.gpsimd.dma_start(out=ff1b_sb, in_=ff1b[:])
        nc.gpsimd.dma_start(out=ident_sb, in_=ident[:])
        gam_bc = bcast("gam_bc", gam)
        cc_bc = bcast("cc_bc", cc)
        nc.sync.dma_start(out=xb_sb[:, 0:2, :], in_=xb[:, 0:2, :])
        nc.sync.dma_start(out=xb_sb[:, 2:4, :], in_=xb[:, 2:4, :])
        nc.scalar.dma_start(out=rw_sb, in_=rw[:])
        nc.scalar.dma_start(out=posT_sb[:, 0:4, :, :], in_=posT[:, 0:4, :, :])
        nc.scalar.dma_start(out=vw_sb, in_=vw[:])
        nc.scalar.dma_start(out=posT_sb[:, 4:NT, :, :],
                            in_=posT[:, 4:NT, :, :])
        nc.sync.dma_start(out=xb_sb[:, 4:NT, :], in_=xb[:, 4:NT, :])
        # rel-shift pad column: exp(0)=1 for exp-product heads, 0 for
        # identity-add heads (additive identity)
        for h, pad in enumerate(pads):
            nc.gpsimd.dma_start(
                out=bass.AP(tensor=pad, offset=0,
                            ap=[[K + 1, 128], [(K + 1) * 128, NT]]),
                in_=zero_row if h in ID_HEADS else ones_row)

        def ln1_stats(t):
            stats = stat_pool.tile([128, 2, 6], F32, tag="stats")
            for g in range(2):
                nc.vector.bn_stats(out=stats[:, g, :],
                                   in_=xb_sb[:, t, g * 512:(g + 1) * 512])
            nc.vector.bn_aggr(out=mv_all[:, t, :], in_=stats)

        def ln1_sqrt(t0, t1):
            nc.scalar.activation(
                out=rstds[:, t0:t1], in_=mv_all[:, t0:t1, 1],
                func=mybir.ActivationFunctionType.Sqrt,
                bias=eps_sb, scale=1.0)
            nc.vector.reciprocal(out=rstds[:, t0:t1], in_=rstds[:, t0:t1])

        def ln1_norm(t):
            # xhat only; gamma/beta folded into the projection weights
            nc.vector.tensor_scalar(out=w_sb[:, t, :], in0=xb_sb[:, t, :],
                                    scalar1=mv_all[:, t, 0:1],
                                    scalar2=rstds[:, t:t + 1],
                                    op0=mybir.AluOpType.subtract,
                                    op1=mybir.AluOpType.mult)
            wt_t = wtp.tile([128, ND, 128], BF16, tag="wt")
            nc.sync.dma_start_transpose(wt_t, w_sb[:, t, :])
            nc.scalar.copy(out=wTf8[:, t, :, :], in_=wt_t)

        def vproj(t):
            pv = ps_rr((psA, psA, psB, psC))
            for dp in range(ND // 2):
                nc.tensor.matmul(
                    pv[:, 0:512], wTf8[:, t, 2 * dp:2 * dp + 2, :],
                    vw_sb[:, 2 * dp:2 * dp + 2, :],
                    start=(dp == 0), stop=(dp == ND // 2 - 1), perf_mode=DR)
            nc.vector.tensor_copy(
                out=v1[:, t, :, 0:DH],
                in_=pv[:, 0:512].rearrange("p (h d) -> p h d", h=HC))

        def projT_fc(w_fc, dst, fc, bias1=None, dst2=None, bias1b=None,
                     rhs_sb=None, pool=None, grp=None, act_copy=False):
            # dst[., fc, i] = (W[:, fc-block].T @ w^T)[feat, tok], DoubleRow
            # over dc pairs.  grp: None = both column groups, "a" = tokens
            # 0:512 (tiles 0-3), "b" = tokens 512:896 (tiles 4-6)
            ps = ps_rr(pool) if isinstance(pool, tuple) else \
                (pool or psB).tile([128, 1024], F32, tag="s")
            full = w_fc.shape[-1] == FC
            for dp in range(ND // 2):
                st = dict(start=(dp == 0), stop=(dp == ND // 2 - 1))
                lh = (w_fc[:, 2 * dp:2 * dp + 2, fc * 128:(fc + 1) * 128]
                      if full else w_fc[:, 2 * dp:2 * dp + 2, :])
                r = rhs_sb if rhs_sb is not None else wTf8
                if grp != "b":
                    nc.tensor.matmul(
                        ps[:, 0:512],
                        lh, r[:, 0:4, 2 * dp:2 * dp + 2, :]
                        .rearrange("p t c f -> p c t f"),
                        perf_mode=DR, **st)
                if grp != "a":
                    nc.tensor.matmul(
                        ps[:, 512:896],
                        lh, r[:, 4:7, 2 * dp:2 * dp + 2, :]
                        .rearrange("p t c f -> p c t f"),
                        perf_mode=DR, **st)
            c0, c1 = (0, K) if grp is None else \
                ((0, 512) if grp == "a" else (512, K))
            if bias1 is not None:
                nc.vector.tensor_scalar_add(
                    out=dst[:, fc, c0:c1], in0=ps[:, c0:c1],
                    scalar1=bias1[:, fc:fc + 1])
                nc.vector.tensor_scalar_add(
                    out=dst2[:, fc, c0:c1], in0=ps[:, c0:c1],
                    scalar1=bias1b[:, fc:fc + 1])
            elif act_copy:
                nc.scalar.copy(out=dst[:, fc, c0:c1], in_=ps[:, c0:c1])
            else:
                nc.vector.tensor_copy(out=dst[:, fc, c0:c1],
                                      in_=ps[:, c0:c1])

        # LN1 pipeline: tiles 0-3 first, then rT group a fills the PE
        # while xb[4:7]/LN-b complete
        ln1_stats(0)
        ln1_stats(1)
        ln1_sqrt(0, 2)
        ln1_norm(0)
        ln1_norm(1)
        ln1_stats(2)
        ln1_stats(3)
        ln1_sqrt(2, 4)
        ln1_norm(2)
        ln1_norm(3)
        for fc in range(NFC):
            projT_fc(rw_sb, rT, fc, rhs_sb=posT_sb,
                     pool=(psA, psA, psB, psC), grp="a",
                     act_copy=(fc % 2 == 0))
        for t in range(4):
            vproj(t)
        for t in range(4, NT):
            ln1_stats(t)
        ln1_sqrt(4, NT)
        for t in range(4, NT):
            ln1_norm(t)
        for fc in range(NFC):
            projT_fc(rw_sb, rT, fc, rhs_sb=posT_sb,
                     pool=(psA, psA, psB, psC), grp="b",
                     act_copy=(fc % 2 == 0))
        for t in range(4, NT):
            vproj(t)
        p0a_stack.close()

        # ---------------- P1: attention ----------------
        bd_pool = attn_stack.enter_context(tc.tile_pool(name="bdp", bufs=2))
        sbdT_pool = attn_stack.enter_context(
            tc.tile_pool(name="sbdp", bufs=2))
        pT_pool = attn_stack.enter_context(tc.tile_pool(name="ptp", bufs=2))
        eac_pool = attn_stack.enter_context(tc.tile_pool(name="eacp", bufs=2))
        rden_pool = attn_stack.enter_context(tc.tile_pool(name="rdnp", bufs=2))

        def bd_phase(h):
            fc, hp = h // 2, (h % 2) * DH
            id_mode = h in ID_HEADS
            bd_all = bd_pool.tile([128, NT, K], BF16, tag="bd")
            for it in range(NT):
                ps = psA.tile([128, 1024], F32, tag="s")
                lh = _dr0(q2T[hp:hp + DH, fc, it * 128:(it + 1) * 128])
                nc.tensor.matmul(ps[:, 0:512],
                                 lh, _dr0(rT[hp:hp + DH, fc, 0:512]),
                                 start=True, stop=True, perf_mode=DR)
                nc.tensor.matmul(ps[:, 512:896],
                                 lh, _dr0(rT[hp:hp + DH, fc, 512:896]),
                                 start=True, stop=True, perf_mode=DR)
                if id_mode:
                    # raw 128x-scaled BD; copy alternates DVE/Act
                    if it % 2 == 0:
                        nc.vector.tensor_copy(out=bd_all[:, it, :],
                                              in_=ps[:, 0:K])
                    else:
                        nc.scalar.copy(out=bd_all[:, it, :], in_=ps[:, 0:K])
                else:
                    nc.scalar.activation(out=bd_all[:, it, :], in_=ps[:, 0:K],
                                         func=mybir.ActivationFunctionType.Exp,
                                         scale=EXPS)
            # pads split 5+2 tiles so the first sbdT transpose half
            # (reads rows <= 640) can start before the tail tiles land
            nc.gpsimd.dma_start(
                out=bass.AP(tensor=pads[h], offset=1,
                            ap=[[K + 1, 128], [(K + 1) * 128, 5], [1, K]]),
                in_=bd_all[:, 0:5, :])
            return nc.gpsimd.dma_start(
                out=bass.AP(tensor=pads[h], offset=(K + 1) * 128 * 5 + 1,
                            ap=[[K + 1, 128], [(K + 1) * 128, 2], [1, K]]),
                in_=bd_all[:, 5:NT, :])

        def st_phase(h):
            # sbdT[j, jt, i] = shifted BD term [i, jt*128+j] via DMA
            # transpose of the padded buffer read at flat offset K + K*i + j.
            fc, hp = h // 2, (h % 2) * DH
            id_mode = h in ID_HEADS
            sbdT = sbdT_pool.tile([128, NT, K], BF16, tag="sbdT")
            tp = nc.sync.dma_start_transpose(
                sbdT, bass.AP(tensor=pads[h], offset=K,
                              ap=[[K, K], [1, K]]))
            pT_all = pT_pool.tile([128, NT, K], BF16, tag="pT")
            for jt in range(NT):
                ps = psA.tile([128, 1024], F32, tag="s")
                lh = _dr0(kT[hp:hp + DH, fc, jt * 128:(jt + 1) * 128])
                if id_mode:
                    nc.tensor.matmul(ps[:, 0:512], lh,
                                     _dr0(q1T[hp:hp + DH, fc, 0:512]),
                                     start=True, stop=False, perf_mode=DR)
                    nc.tensor.matmul(ps[:, 512:896], lh,
                                     _dr0(q1T[hp:hp + DH, fc, 512:896]),
                                     start=True, stop=False, perf_mode=DR)
                    nc.tensor.matmul(ps[:, 0:512], ident_sb,
                                     sbdT[:, jt, 0:512],
                                     start=False, stop=True)
                    nc.tensor.matmul(ps[:, 512:896], ident_sb,
                                     sbdT[:, jt, 512:896],
                                     start=False, stop=True)
                else:
                    nc.tensor.matmul(ps[:, 0:512], lh,
                                     _dr0(q1T[hp:hp + DH, fc, 0:512]),
                                     start=True, stop=True, perf_mode=DR)
                    nc.tensor.matmul(ps[:, 512:896], lh,
                                     _dr0(q1T[hp:hp + DH, fc, 512:896]),
                                     start=True, stop=True, perf_mode=DR)
                nc.scalar.activation(out=pT_all[:, jt, :], in_=ps[:, 0:K],
                                     func=mybir.ActivationFunctionType.Exp,
                                     scale=EXPS)
            if not id_mode:
                for jt in range(NT):
                    nc.vector.tensor_mul(out=pT_all[:, jt, :],
                                         in0=pT_all[:, jt, :],
                                         in1=sbdT[:, jt, :])
            return pT_all, tp

        def pv_phase(h, pT_all):
            fc, hp = h // 2, (h % 2) * DH
            pvx = psC.tile([128, 1024], F32, tag="s")
            for jt in range(NT):
                st = dict(start=(jt == 0), stop=(jt == NT - 1))
                lh = v1[:, jt, h, :]
                nc.tensor.matmul(pvx[0:DH + 1, 0:512], lh,
                                 pT_all[:, jt, 0:512], **st)
                nc.tensor.matmul(pvx[0:DH + 1, 512:896], lh,
                                 pT_all[:, jt, 512:896], **st)
            rden = rden_pool.tile([1, K], BF16, tag="rden")
            with nc.allow_low_precision(reason="softmax denom recip in bf16"):
                nc.vector.reciprocal(out=rden, in_=pvx[DH:DH + 1, 0:K])
            bc = psB.tile([128, 1024], F32, tag="s")
            nc.tensor.matmul(bc[0:DH, 0:512], ones64, rden[:, 0:512],
                             start=True, stop=True)
            nc.tensor.matmul(bc[0:DH, 512:896], ones64, rden[:, 512:896],
                             start=True, stop=True)
            bc_sb = eac_pool.tile([DH, K], BF16, tag="bc_sb")
            nc.vector.tensor_copy(out=bc_sb, in_=bc[0:DH, 0:K])
            nc.vector.tensor_mul(out=avT[hp:hp + DH, fc, :],
                                 in0=pvx[0:DH, 0:K], in1=bc_sb)

        def poa_phase():
            # o_proj partial (fc 0,1 = heads 0..3) + per-core constant
            # (beta + beta@Wv@Wo fold); fills the P1 tail.  ow is x8 in
            # fp8, folded back by the 1/8 in the copy.
            for it in range(NT):
                po = (psB if it % 2 else psC).tile([128, 1024], F32, tag="s")
                l = avT[:, 0:2, it * 128:(it + 1) * 128]
                nc.tensor.matmul(po[:, 0:512], l, ow_sb[:, 0:2, 0:512],
                                 start=True, stop=True, perf_mode=DR)
                nc.tensor.matmul(po[:, 512:1024], l, ow_sb[:, 0:2, 512:1024],
                                 start=True, stop=True, perf_mode=DR)
                nc.vector.scalar_tensor_tensor(
                    out=ao_sb[:, it, :], in0=po, scalar=1.0 / WS,
                    in1=cc_bc, op0=mybir.AluOpType.mult,
                    op1=mybir.AluOpType.add)

        # heads interleave with the q/k projection chunks they depend on;
        # FFN weights stream on the SP queue behind each head's sbdT
        pending = {}
        ffq = ([("ow", c) for c in range(2)]
               + [("ff1", dc) for dc in range(ND)])

        def stream_ffw(n, anchor):
            for _ in range(n):
                if not ffq:
                    return
                kind, c = ffq.pop(0)
                if kind == "ow":
                    d = nc.sync.dma_start(out=ow_sb[:, 2 * c:2 * c + 2, :],
                                          in_=ow[:, 2 * c:2 * c + 2, :])
                else:
                    d = nc.sync.dma_start(out=ff1w_sb[:, c, :],
                                          in_=ff1w[:, c, :])
                if anchor is not None:
                    add_dep_helper(d.ins, anchor.ins, sync=True,
                                   reason="stream ff weights behind P1")

        def load_qk(fc):
            qw_fc = qkp.tile([128, ND, 128], E4, tag="qw")
            nc.gpsimd.dma_start(out=qw_fc, in_=qw[:, fc, :, :])
            kw_fc = qkp.tile([128, ND, 128], E4, tag="kw")
            nc.gpsimd.dma_start(out=kw_fc, in_=kw[:, fc, :, :])
            return qw_fc, kw_fc

        qk_bufs = [load_qk(0), load_qk(1)]
        for fc in range(NFC):
            qw_fc, kw_fc = qk_bufs[fc % 2]
            pp = psB
            projT_fc(qw_fc, q1T, fc, bias1=rwb_sb, dst2=q2T, bias1b=rrb_sb,
                     pool=pp)
            projT_fc(kw_fc, kT, fc, pool=pp, act_copy=True)
            if fc + 2 < NFC:
                qk_bufs[fc % 2] = load_qk(fc + 2)
            h0, h1 = 2 * fc, 2 * fc + 1
            bd_phase(h0)
            if h0 >= PVLAG:
                pv_phase(h0 - PVLAG, pending.pop(h0 - PVLAG)[0])
            bd_phase(h1)
            if h0 < NT:
                nc.vector.tensor_mul(out=w_sb[:, h0, :],
                                     in0=w_sb[:, h0, :], in1=gam_bc)
            pending[h0] = st_phase(h0)
            stream_ffw(2 if h0 < 4 else 1, pending[h0][1])
            if h1 >= PVLAG:
                pv_phase(h1 - PVLAG, pending.pop(h1 - PVLAG)[0])
            if h1 < NT:
                nc.vector.tensor_mul(out=w_sb[:, h1, :],
                                     in0=w_sb[:, h1, :], in1=gam_bc)
            pending[h1] = st_phase(h1)
            stream_ffw(2 if h1 < 4 else 1, pending[h1][1])
        stream_ffw(len(ffq), None)
        for h in range(HC - PVLAG, HC):
            pv_phase(h, pending.pop(h)[0])
        poa_phase()

        attn_stack.close()
        p0_stack.close()

        # ---------------- P2+P3: o_proj 2nd half, exchange, LN2, FFN ----
        p23w = ctx.enter_context(tc.tile_pool(name="p23w", bufs=1))
        ff2w_sb = p23w.tile([128, NFF, D], BF16)
        hT = p23w.tile([128, NFF, K], BF16)
        aor_sb = p23w.tile([128, NT, D], BF16)
        ars_all = aor_sb  # LN2 output overwrites the exchanged ao in place
        arT = p23w.tile([128, NT, ND, 128], BF16)

        with tc.tile_pool(name="p2", bufs=2) as p2:
            for ci, (t0, nt_c) in enumerate(CH):
                ain, aout = ao_bufs[ci]
                if ci == 1:
                    # ff2w streams early on the SP queue (behind chunk 0's
                    # exchange DMAs, ahead of the later chunks')
                    for g in range(4):
                        nc.sync.dma_start(
                            out=ff2w_sb[:, 4 * g:4 * g + 4, :],
                            in_=ff2w[:, 4 * g:4 * g + 4, :])
                for it in range(t0, t0 + nt_c):
                    po = (psB if it % 2 else psC).tile([128, 1024], F32,
                                                       tag="s")
                    l = avT[:, 2:4, it * 128:(it + 1) * 128]
                    nc.tensor.matmul(po[:, 0:512], l, ow_sb[:, 2:4, 0:512],
                                     start=True, stop=True, perf_mode=DR)
                    nc.tensor.matmul(po[:, 512:1024], l,
                                     ow_sb[:, 2:4, 512:1024],
                                     start=True, stop=True, perf_mode=DR)
                    nc.vector.scalar_tensor_tensor(
                        out=ao_sb[:, it, :], in0=po, scalar=1.0 / WS,
                        in1=ao_sb[:, it, :], op0=mybir.AluOpType.mult,
                        op1=mybir.AluOpType.add)
                nc.scalar.dma_start(out=ain[:], in_=ao_sb[:, t0:t0 + nt_c, :])
                if sim_mode:
                    nc.gpsimd.dma_start(out=aout[:], in_=ain[:])
                else:
                    nc.gpsimd.collective_compute(
                        "AllReduce", mybir.AluOpType.add,
                        replica_groups=[[0, 1], [2, 3], [4, 5], [6, 7]],
                        ins=[ain[:]], outs=[aout[:]])
                nc.sync.dma_start(out=aor_sb[:, t0:t0 + nt_c, :], in_=aout[:])
                for it in range(t0, t0 + nt_c):
                    x2 = p2.tile([128, D], BF16, tag="x2")
                    nc.vector.tensor_add(out=x2, in0=w_sb[:, it, :],
                                         in1=aor_sb[:, it, :])
                    stats = stat_pool.tile([128, 2, 6], F32, tag="stats")
                    for g in range(2):
                        nc.vector.bn_stats(out=stats[:, g, :],
                                           in_=x2[:, g * 512:(g + 1) * 512])
                    mv = stat_pool.tile([128, 2], F32, tag="mv")
                    nc.vector.bn_aggr(out=mv, in_=stats)
                    rstd = stat_pool.tile([128, 1], F32, tag="rstd")
                    nc.scalar.activation(
                        out=rstd, in_=mv[:, 1:2],
                        func=mybir.ActivationFunctionType.Sqrt,
                        bias=eps_sb, scale=1.0)
                    nc.vector.reciprocal(out=rstd, in_=rstd)
                    # LN2 affine is folded into ff1w/ff1b on the host
                    nc.vector.tensor_scalar(out=ars_all[:, it, :], in0=x2,
                                            scalar1=mv[:, 0:1], scalar2=rstd,
                                            op0=mybir.AluOpType.subtract,
                                            op1=mybir.AluOpType.mult)
                nc.sync.dma_start_transpose(arT[:, t0:t0 + nt_c, :, :],
                                            ars_all[:, t0:t0 + nt_c, :])
                # FFN1 for this chunk
                n = nt_c * 128
                for ffc in range(NFF):
                    ph = psA.tile([128, 1024], F32, tag="s")
                    for dc in range(ND):
                        st = dict(start=(dc == 0), stop=(dc == ND - 1))
                        lh = ff1w_sb[:, dc, ffc * 128:(ffc + 1) * 128]
                        nc.tensor.matmul(ph[:, 0:n], lh,
                                         arT[:, t0:t0 + nt_c, dc, :], **st)
                    nc.scalar.activation(
                        out=hT[:, ffc, t0 * 128:t0 * 128 + n], in_=ph[:, 0:n],
                        func=mybir.ActivationFunctionType.Relu,
                        bias=ff1b_sb[:, ffc:ffc + 1], scale=1.0)

        # ---------------- FFN2 ----------------
        with tc.tile_pool(name="p3", bufs=3) as p3:
            for it in range(NT):
                po = (psB if it % 2 else psC).tile([128, 1024], F32, tag="s")
                if it < NT - 1:
                    for ffc in range(NFF):
                        st = dict(start=(ffc == 0), stop=(ffc == NFF - 1))
                        l = hT[:, ffc, it * 128:(it + 1) * 128]
                        nc.tensor.matmul(po[:, 0:512], l,
                                         ff2w_sb[:, ffc, 0:512], **st)
                        nc.tensor.matmul(po[:, 512:1024], l,
                                         ff2w_sb[:, ffc, 512:1024], **st)
                    ot = p3.tile([128, D], BF16, tag="ot")
                    nc.vector.tensor_copy(out=ot, in_=po)
                    nc.sync.dma_start(out=out[:, it, :], in_=ot)
                else:
                    # last tile: accumulate+drain per quarter so early
                    # quarters' copy+DMA overlap later quarters' matmuls
                    for qi, c0 in enumerate((0, 256, 512, 768)):
                        poq = psA.tile([128, 1024], F32, tag="s")
                        for ffc in range(NFF):
                            st = dict(start=(ffc == 0), stop=(ffc == NFF - 1))
                            l = hT[:, ffc, it * 128:(it + 1) * 128]
                            nc.tensor.matmul(poq[:, 0:256], l,
                                             ff2w_sb[:, ffc, c0:c0 + 256],
                                             **st)
                        oth = p3.tile([128, 256], BF16, tag="oth")
                        nc.vector.tensor_copy(out=oth, in_=poq[:, 0:256])
                        nc.sync.dma_start(out=out[:, it, c0:c0 + 256],
                                          in_=oth)

    nc.finalize()
    return nc


def prep_inputs(inputs):
    """Full inputs -> list of 8 per-core input maps."""
    x_nat = np.concatenate([inputs["input_ids"], inputs["mem"],
                            inputs["c_mem"]], axis=0)  # [K,B,D] f32

    def to8(x, scale=1.0):
        return np.clip(np.asarray(x, np.float32) * scale,
                       -240, 240).astype(E4NP)

    # posT[p, t, dc, l] = pos[t*128 + l, dc*128 + p], fp8
    posn = np.asarray(inputs["positional_embedding"], np.float32)
    posT_t = np.ascontiguousarray(
        to8(posn).reshape(NT, 128, ND, 128).transpose(3, 0, 2, 1))

    qkv = np.asarray(inputs["qkv_w"], np.float32)
    gamma = np.asarray(inputs["ln1_w"], np.float32)
    beta = np.asarray(inputs["ln1_b"], np.float32)
    # fold LN1 affine: w = xhat*gamma + beta
    Wq = qkv[:, 0 * H * DH:1 * H * DH]
    Wk = qkv[:, 1 * H * DH:2 * H * DH]
    Wv = qkv[:, 2 * H * DH:3 * H * DH]
    Wq_g, Wk_g, Wv_g = (gamma[:, None] * Wq, gamma[:, None] * Wk,
                        gamma[:, None] * Wv)
    cq = beta @ Wq          # [H*DH] folds into both q biases
    cv = beta @ Wv          # [H*DH] folds into the o-proj constant
    # beta @ Wk adds a per-row constant to the logits -> cancels in softmax
    rwb_eff = np.asarray(inputs["r_w_bias"], np.float32).reshape(-1) + cq
    rrb_eff = np.asarray(inputs["r_r_bias"], np.float32).reshape(-1) + cq

    # fold LN2's affine into the FFN first layer
    ln2w_f = np.asarray(inputs["ln2_w"], np.float32)
    ln2b_f = np.asarray(inputs["ln2_b"], np.float32)
    ff1w_eff = np.asarray(inputs["ff1_w"], np.float32) * ln2w_f[:, None]
    ff1b_eff = (np.asarray(inputs["ff1_b"], np.float32)
                + ln2b_f @ np.asarray(inputs["ff1_w"], np.float32))
    o_w = np.asarray(inputs["o_w"], np.float32)
    ident_np = np.eye(128, dtype=np.float32).astype(BF)
    maps = []
    for c in range(8):
        b, hh = c // 2, c % 2
        Fs = slice(hh * FC, (hh + 1) * FC)
        FFs = slice(hh * FFC, (hh + 1) * FFC)
        # per-core o-proj constant: cv@Wo for this half + beta/2 (the
        # pairwise allreduce sums both halves -> cv@Wo full + beta)
        cc_core = cv[Fs] @ o_w[Fs, :] + 0.5 * beta

        def wchunk8(wmat):  # [D, FC] -> [128, ND, FC] fp8 x8
            return np.ascontiguousarray(
                to8(wmat, WS).reshape(ND, 128, -1).transpose(1, 0, 2))

        def wchunk(wmat):   # [D, F] -> [128, ND, F] bf16
            return np.ascontiguousarray(
                wmat.astype(BF).reshape(ND, 128, -1).transpose(1, 0, 2))

        m = {
            "xb": np.ascontiguousarray(
                x_nat[:, b, :].astype(BF).reshape(NT, 128, D)
                .transpose(1, 0, 2)),
            "posT": posT_t,
            # fc-major fp8: [128, NFC, ND, 128]
            "qw": np.ascontiguousarray(
                wchunk8(Wq_g[:, Fs]).reshape(128, ND, NFC, 128)
                .transpose(0, 2, 1, 3)),
            "kw": np.ascontiguousarray(
                wchunk8(Wk_g[:, Fs]).reshape(128, ND, NFC, 128)
                .transpose(0, 2, 1, 3)),
            "vw": wchunk8(Wv_g[:, Fs]),
            "rw": wchunk8(np.asarray(inputs["r_w"], np.float32)[:, Fs]),
            "ow": np.ascontiguousarray(
                to8(o_w[Fs, :], WS)
                .reshape(NFC, 128, D).transpose(1, 0, 2)),
            "ff1w": wchunk(ff1w_eff[:, FFs]),
            "ff2w": np.ascontiguousarray(
                np.asarray(inputs["ff2_w"], np.float32)[FFs, :].astype(BF)
                .reshape(NFF, 128, D).transpose(1, 0, 2)),
            # q biases are added to the 8x-scaled q: upload x8
            "rwb": np.ascontiguousarray(
                (WS * rwb_eff[hh * FC:(hh + 1) * FC])
                .reshape(NFC, 128).T.astype(np.float32)),
            "rrb": np.ascontiguousarray(
                (WS * rrb_eff[hh * FC:(hh + 1) * FC])
                .reshape(NFC, 128).T.astype(np.float32)),
            "ff1b": np.ascontiguousarray(
                ff1b_eff[FFs].reshape(NFF, 128).T.astype(np.float32)),
            "gam": gamma.astype(BF),
            "cc": cc_core.astype(BF),
            "ident": ident_np,
        }
        maps.append(m)
    return maps


class PjrtRunner:
    """Persistent jitted SPMD executor for a prebuilt Bass module."""

    def __init__(self, nc, n_cores=8):
        import jax
        from jax.sharding import Mesh, PartitionSpec
        from jax.experimental.shard_map import shard_map
        from concourse import mybir as _mybir
        from concourse.bass2jax import (_bass_exec_p, install_neuronx_cc_hook,
                                        partition_id_tensor)
        install_neuronx_cc_hook()
        self.jax = jax
        self.n_cores = n_cores
        in_names, out_names, out_avals = [], [], []
        partition_name = (nc.partition_id_tensor.name
                          if nc.partition_id_tensor else None)
        for alloc in nc.m.functions[0].allocations:
            if not isinstance(alloc, _mybir.MemoryLocationSet):
                continue
            name = alloc.memorylocations[0].name
            if alloc.kind == "ExternalInput":
                if name != partition_name:
                    in_names.append(name)
            elif alloc.kind == "ExternalOutput":
                out_names.append(name)
                out_avals.append(jax.core.ShapedArray(
                    tuple(alloc.tensor_shape), _mybir.dt.np(alloc.dtype)))
        self.in_names, self.out_names, self.out_avals = \
            in_names, out_names, out_avals

        def _body(*args):
            operands = list(args)
            if partition_name is not None:
                operands.append(partition_id_tensor())
            all_in = in_names + out_names
            if partition_name is not None:
                all_in = all_in + [partition_name]
            return tuple(_bass_exec_p.bind(
                *operands,
                out_avals=tuple(out_avals),
                in_names=tuple(all_in),
                out_names=tuple(out_names),
                lowering_input_output_aliases=(),
                sim_require_finite=True,
                sim_require_nnan=True,
                nc=nc,
            ))

        devices = jax.devices()[:n_cores]
        self.mesh = Mesh(np.asarray(devices), ("core",))
        nin = len(in_names) + len(out_names)
        self.fn = jax.jit(shard_map(
            _body, mesh=self.mesh,
            in_specs=(PartitionSpec("core"),) * nin,
            out_specs=(PartitionSpec("core"),) * len(out_names),
            check_rep=False))

    def pack(self, maps):
        arrs = [self.jax.device_put(
                    np.concatenate([np.asarray(maps[c][n])
                                    for c in range(self.n_cores)], axis=0))
                for n in self.in_names]
        arrs += [self.jax.device_put(
                    np.zeros((self.n_cores * a.shape[0], *a.shape[1:]),
                             a.dtype))
                 for a in self.out_avals]
        return arrs

    def __call__(self, packed):
        return self.fn(*packed)

    def unpack(self, outs):
        res = []
        for c in range(self.n_cores):
            res.append({
                n: np.asarray(outs[i]).reshape(
                    self.n_cores, *self.out_avals[i].shape)[c]
                for i, n in enumerate(self.out_names)})
        return res


def get_runner(debug=False, sim_mode=False):
    key = (bool(debug), bool(sim_mode))
    if key not in _CACHED:
        nc = build(debug=debug, sim_mode=sim_mode)
        _CACHED[key] = PjrtRunner(nc, 8)
    return _CACHED[key]


def _unpack_out(arr):
    # [128, NT, D] -> [K, D]
    return np.asarray(arr, np.float32).transpose(1, 0, 2).reshape(K, D)


def _assemble(inputs, results):
    ff2b = np.asarray(inputs["ff2_b"], np.float32)
    out = np.zeros((K, B, D), np.float32)
    for b in range(B):
        out[:, b, :] = (_unpack_out(results[2 * b]["out"])
                        + _unpack_out(results[2 * b + 1]["out"])
                        + ff2b[None, :])
    return out


def run(inputs, trace=False, debug=False):
    runner = get_runner(debug=False)
    maps = prep_inputs(inputs)
    packed = runner.pack(maps)
    outs = runner(packed)
    results = runner.unpack(outs)

    class R:
        pass
    res = R()
    res.results = results
    res.exec_time_ns = None
    return _assemble(inputs, results), res


def kernel(**inputs):
    inputs = {k: np.asarray(v) for k, v in inputs.items()}
    out, _ = run(inputs, trace=False, debug=False)
    return out
